# revision 16
# baseline (speedup 1.0000x reference)
"""Distributed multi-head attention kernel for 8 TRN2 NeuronCores.

Module: B=2, N=2048, D_MODEL=1024, H=16, D_HEAD=64 attention with
arbitrary (non-paired-frequency) rotary embedding, key-side boolean
masking, softmax, and output projection.

Sharding: head-parallel attention (2 heads per core, both batches),
then one AllToAll (2 MB/core) to switch to row-parallel for the output
projection. Each core returns a [512, 1024] row block of the output.

Math layout notes:
 - All matmuls run in float32r (TF32-like, ~1e-4 rel err, 4x faster
   than fp32 on the PE).
 - qT/kT are produced directly in [chan, row] layout (chan on
   partitions) so scores = kT_tile.T @ qT come out as
   scoresT [keys, qrows] with keys on partitions.
 - Rotary: rotate_every_two(x @ W) == x @ Wr with Wr a column
   permutation/negation of W (host-prepped), so
   q' = (x@Wq)*cos + (x@Wqr)*sin — two matmul chains + 3 DVE ops.
 - Key masking is folded into the softmax exp as a per-partition
   (per-key) bias: exp(0.125*score + (mask ? 0 : -1e5)).
 - Softmax denominator comes from a ones-column appended to V
   (lhsT = [v_head | 1], M=65): PV accumulates [65, qrows] where row 64
   is sum_j p[j, i].
"""
import os
import warnings

warnings.filterwarnings("ignore")
import numpy as np

from concourse import bacc, tile, mybir, bass_utils


def _install_trace_shim():
    """Register an antenv.axon_hooks NTFF-profile shim (missing in this
    image) so run_bass_kernel_spmd(trace=True) can capture HW profiles."""
    import sys
    import types
    import ctypes
    import contextlib

    if "antenv.axon_hooks" in sys.modules:
        return
    so_path = "/opt/axon/libaxon_pjrt.so"
    hook = None
    if os.path.exists(so_path):
        lib = ctypes.CDLL(so_path)
        if hasattr(lib, "axon_start_nrt_profile"):
            lib.axon_start_nrt_profile.argtypes = [
                ctypes.POINTER(ctypes.c_int64), ctypes.c_size_t]
            lib.axon_start_nrt_profile.restype = ctypes.c_int64
            lib.axon_stop_nrt_profile.argtypes = [ctypes.c_char_p]
            lib.axon_stop_nrt_profile.restype = ctypes.c_int64

            @contextlib.contextmanager
            def _hook(output_dir, device_ids):
                import jax
                jax.devices()
                if device_ids:
                    ids = (ctypes.c_int64 * len(device_ids))(*device_ids)
                    rc = lib.axon_start_nrt_profile(ids, len(device_ids))
                else:
                    rc = lib.axon_start_nrt_profile(None, 0)
                if rc != 0:
                    raise RuntimeError(f"axon_start_nrt_profile rc={rc}")
                try:
                    yield
                finally:
                    n = lib.axon_stop_nrt_profile(str(output_dir).encode())
                    print(f"[trace] {n} profile file(s) -> {output_dir}")

            hook = _hook

    mod = types.ModuleType("antenv.axon_hooks")
    mod.get_axon_ntff_profile_hook = lambda: hook
    mod.set_axon_ntff_profile_hook = lambda h: None
    sys.modules["antenv.axon_hooks"] = mod
    bass_utils.upload_artifacts = lambda tmpdir: tmpdir

B, N, DM, H, DH = 2, 2048, 1024, 16, 64
R = B * N            # 4096 flattened rows
NCORES = 8
HPC = 2              # heads per core
CPC = HPC * DH       # chans per core = 128
KT = 8               # contraction tiles over d_model
RB = 8               # row blocks of 512 over R
QB = 4               # q blocks of 512 over N
NKEYT = 16           # key tiles of 128 over N
ROWS_PER_CORE = R // NCORES  # 512

F32 = mybir.dt.float32
F32R = mybir.dt.float32r

LAST_EXEC_TIME_NS = None
LAST_TRACE_DIR = None


def _rot_cols(w):
    """rotate_every_two on output columns: wr[:,2i] = -w[:,2i+1], wr[:,2i+1] = w[:,2i]."""
    wr = np.empty_like(w)
    wr[:, 0::2] = -w[:, 1::2]
    wr[:, 1::2] = w[:, 0::2]
    return wr


def build(dbg=False):
    nc = bacc.Bacc("TRN2", target_bir_lowering=False, debug=False,
                   num_devices=NCORES)

    xt_d = nc.dram_tensor("xt", [DM, R], F32R, kind="ExternalInput")
    wq_d = nc.dram_tensor("wq", [DM, CPC], F32R, kind="ExternalInput")
    wqr_d = nc.dram_tensor("wqr", [DM, CPC], F32R, kind="ExternalInput")
    wk_d = nc.dram_tensor("wk", [DM, CPC], F32R, kind="ExternalInput")
    wkr_d = nc.dram_tensor("wkr", [DM, CPC], F32R, kind="ExternalInput")
    wv_d = nc.dram_tensor("wv", [DM, CPC], F32R, kind="ExternalInput")
    wout_d = nc.dram_tensor("wout", [DM, DM], F32R, kind="ExternalInput")
    boutb_d = nc.dram_tensor("boutb", [128, DM], F32, kind="ExternalInput")
    cost_d = nc.dram_tensor("cost", [CPC, N], F32, kind="ExternalInput")
    sint_d = nc.dram_tensor("sint", [CPC, N], F32, kind="ExternalInput")
    maskb_d = nc.dram_tensor("maskb", [128, R // 128], F32, kind="ExternalInput")
    vones_d = nc.dram_tensor("vones", [128, (R // 128) * 2], F32R,
                             kind="ExternalInput")

    out_d = nc.dram_tensor("out", [ROWS_PER_CORE, DM], F32, kind="ExternalOutput")
    if dbg:
        dbg_qt = nc.dram_tensor("dbg_qt", [CPC, R], F32R, kind="ExternalOutput")
        dbg_kt = nc.dram_tensor("dbg_kt", [CPC, R], F32R, kind="ExternalOutput")
        dbg_vaug = nc.dram_tensor("dbg_vaug", [128, (R // 128) * 2 * (DH + 1)],
                                  F32R, kind="ExternalOutput")
        dbg_p = nc.dram_tensor("dbg_p", [128, N], F32R, kind="ExternalOutput")
        dbg_on = nc.dram_tensor("dbg_on", [DH, 4 * N], F32R, kind="ExternalOutput")
        dbg_oall = nc.dram_tensor("dbg_oall", [128, KT * 512], F32R,
                                  kind="ExternalOutput")

    a2a_in = nc.dram_tensor("a2a_in", [DM, ROWS_PER_CORE], F32R)
    a2a_out = nc.dram_tensor("a2a_out", [DM, ROWS_PER_CORE], F32R)

    # [DM, cols] viewed as [128, KT, cols] (contraction-tile-major)
    def ktview(d, cols):
        return d.ap().rearrange("(k p) n -> p k n", p=128)

    VAUGW = 2 * (DH + 1)  # 130 cols per key tile: [vA | 1 | vB | 1]

    with tile.TileContext(nc) as tc:
        with tc.tile_pool(name="persist", bufs=1) as pp:
            # persistent tiles
            wq_sb = pp.tile([128, KT, CPC], F32R, tag="wq")
            wqr_sb = pp.tile([128, KT, CPC], F32R, tag="wqr")
            wk_sb = pp.tile([128, KT, CPC], F32R, tag="wk")
            wkr_sb = pp.tile([128, KT, CPC], F32R, tag="wkr")
            wv_sb = pp.tile([128, KT, CPC], F32R, tag="wv")
            cost_sb = pp.tile([CPC, N], F32, tag="cost")
            sint_sb = pp.tile([CPC, N], F32, tag="sint")
            maskb_sb = pp.tile([128, R // 128], F32, tag="maskb")
            boutb_sb = pp.tile([128, DM], F32, tag="boutb")
            qt_sb = pp.tile([CPC, R], F32R, tag="qt")
            kt_sb = pp.tile([CPC, R], F32R, tag="kt")
            vaug_sb = pp.tile([128, (R // 128) * VAUGW], F32R, tag="vaug")

            for sb_t, d_t in [(wq_sb, wq_d), (wqr_sb, wqr_d), (wk_sb, wk_d),
                              (wkr_sb, wkr_d), (wv_sb, wv_d)]:
                nc.sync.dma_start(sb_t[:], ktview(d_t, CPC))
            nc.sync.dma_start(cost_sb[:], cost_d[:, :])
            nc.sync.dma_start(sint_sb[:], sint_d[:, :])
            nc.sync.dma_start(maskb_sb[:], maskb_d[:, :])
            nc.sync.dma_start(boutb_sb[:], boutb_d[:, :])
            # ones columns for the v_aug denominator trick: cols kt*130+64
            # and kt*130+129 of vaug (DVE memset has no f32r encoding, so
            # DMA host-provided ones into the strided view)
            ones_view = vaug_sb[:].rearrange("p (t u w) -> p (t u) w",
                                             u=2, w=DH + 1)[:, :, DH]
            nc.sync.dma_start(ones_view, vones_d[:, :])

            xt_view = xt_d.ap().rearrange("(k p) n -> p k n", p=128)

            # ---- Phase 1: projections + rotary + v_aug ----
            with tc.tile_pool(name="p1", bufs=2) as p1, \
                 tc.tile_pool(name="ps1", bufs=1, space="PSUM") as ps1:
                for rb in range(RB):
                    c0 = rb * 512
                    xt_sb = p1.tile([128, KT, 512], F32R, tag="xt")
                    nc.sync.dma_start(xt_sb[:], xt_view[:, :, c0:c0 + 512])

                    q_ps = ps1.tile([128, 512], F32, tag="q")
                    qr_ps = ps1.tile([128, 512], F32, tag="qr")
                    k_ps = ps1.tile([128, 512], F32, tag="k")
                    kr_ps = ps1.tile([128, 512], F32, tag="kr")
                    v_ps = ps1.tile([128, 512], F32, tag="v")
                    for kt in range(KT):
                        st, sp = kt == 0, kt == KT - 1
                        nc.tensor.matmul(q_ps[:], wq_sb[:, kt, :], xt_sb[:, kt, :],
                                         start=st, stop=sp)
                        nc.tensor.matmul(qr_ps[:], wqr_sb[:, kt, :], xt_sb[:, kt, :],
                                         start=st, stop=sp)
                        nc.tensor.matmul(k_ps[:], wk_sb[:, kt, :], xt_sb[:, kt, :],
                                         start=st, stop=sp)
                        nc.tensor.matmul(kr_ps[:], wkr_sb[:, kt, :], xt_sb[:, kt, :],
                                         start=st, stop=sp)
                        # v in row-major: [rows, chan] via lhsT = xT chunk.
                        # start=True clears the whole PSUM bank, so only the
                        # FIRST matmul touching this bank may carry it.
                        for vt in range(4):
                            nc.tensor.matmul(
                                v_ps[:, vt * 128:(vt + 1) * 128],
                                xt_sb[:, kt, vt * 128:(vt + 1) * 128],
                                wv_sb[:, kt, :], start=(st and vt == 0), stop=sp)

                    # rotary: q' = q*cos + qrot*sin (cos/sin indexed mod N)
                    cc = c0 % N
                    tmp = p1.tile([128, 512], F32R, tag="rottmp")
                    for dst, a_ps, b_ps in [(qt_sb, q_ps, qr_ps),
                                            (kt_sb, k_ps, kr_ps)]:
                        dv = dst[:, c0:c0 + 512]
                        nc.vector.tensor_mul(dv, a_ps[:], cost_sb[:, cc:cc + 512])
                        nc.vector.tensor_mul(tmp[:], b_ps[:], sint_sb[:, cc:cc + 512])
                        nc.vector.tensor_add(dv, dv, tmp[:])

                    # v_aug: copy v rows into [vA | 1 | vB | 1] layout
                    # (4 key tiles per row block)
                    kt0 = rb * 4
                    va = vaug_sb[:].rearrange("p (t w) -> p t w", w=VAUGW)
                    vp = v_ps[:].rearrange("p (t c) -> p t c", c=128)
                    nc.vector.tensor_copy(va[:, kt0:kt0 + 4, 0:DH],
                                          vp[:, :, 0:DH])
                    nc.vector.tensor_copy(va[:, kt0:kt0 + 4, DH + 1:DH + 1 + DH],
                                          vp[:, :, DH:2 * DH])

            if dbg:
                nc.sync.dma_start(dbg_qt[:, :], qt_sb[:])
                nc.sync.dma_start(dbg_kt[:, :], kt_sb[:])
                nc.sync.dma_start(dbg_vaug[:, :], vaug_sb[:])

            # ---- Phase 2: attention per (batch, head) ----
            with tc.tile_pool(name="p2", bufs=2) as p2, \
                 tc.tile_pool(name="ps_sc", bufs=1, space="PSUM") as ps_sc, \
                 tc.tile_pool(name="ps_o", bufs=1, space="PSUM") as ps_o:
                for b in range(B):
                    for h in range(HPC):
                        ho = h * DH
                        bo = b * N
                        o_ps = ps_o.tile([DH + 1, N], F32, tag="outp")
                        for kt in range(NKEYT):
                            g = b * NKEYT + kt          # global key tile
                            krow = bo + kt * 128
                            sc_ps = ps_sc.tile([128, N], F32, tag="sc")
                            for qb in range(QB):
                                nc.tensor.matmul(
                                    sc_ps[:, qb * 512:(qb + 1) * 512],
                                    kt_sb[ho:ho + DH, krow:krow + 128],
                                    qt_sb[ho:ho + DH, bo + qb * 512:bo + (qb + 1) * 512],
                                    start=True, stop=True)
                            p_sb = p2.tile([128, N], F32R, tag="p")
                            nc.scalar.activation(
                                p_sb[:], sc_ps[:], mybir.ActivationFunctionType.Exp,
                                bias=maskb_sb[:, g:g + 1], scale=float(DH ** -0.5))
                            if dbg and b == 0 and h == 0 and kt == 0:
                                nc.sync.dma_start(dbg_p[:, :], p_sb[:])
                            va_l = vaug_sb[:, g * VAUGW + h * (DH + 1):
                                           g * VAUGW + h * (DH + 1) + DH + 1]
                            for qb in range(QB):
                                nc.tensor.matmul(
                                    o_ps[:, qb * 512:(qb + 1) * 512],
                                    va_l,
                                    p_sb[:, qb * 512:(qb + 1) * 512],
                                    start=(kt == 0), stop=(kt == NKEYT - 1))

                        # normalize: out[d, i] / out[64, i]
                        recip = p2.tile([1, N], F32, tag="recip")
                        nc.vector.reciprocal(recip[:], o_ps[DH:DH + 1, :])
                        recipb = p2.tile([DH, N], F32, tag="recipb")
                        nc.gpsimd.partition_broadcast(recipb[:], recip[:])
                        onorm = p2.tile([DH, N], F32R, tag="onorm")
                        nc.vector.tensor_mul(onorm[:], o_ps[0:DH, :], recipb[:])

                        if dbg:
                            nc.sync.dma_start(
                                dbg_on[:, (b * HPC + h) * N:(b * HPC + h + 1) * N],
                                onorm[:])
                        # scatter to A2A source: shard j gets my chans for
                        # global row block j
                        for jj in range(4):
                            j = b * 4 + jj
                            nc.sync.dma_start(
                                a2a_in[j * CPC + ho: j * CPC + ho + DH, :],
                                onorm[:, jj * 512:(jj + 1) * 512])

            # ---- Phase 3: AllToAll + output projection ----
            nc.gpsimd.collective_compute(
                "AllToAll", mybir.AluOpType.bypass,
                replica_groups=[list(range(NCORES))],
                ins=[a2a_in.ap().opt()],
                outs=[a2a_out.ap().opt()])

            with tc.tile_pool(name="p3", bufs=2) as p3, \
                 tc.tile_pool(name="ps3", bufs=2, space="PSUM") as ps3:
                oall_sb = p3.tile([128, KT, 512], F32R, tag="oall")
                nc.sync.dma_start(oall_sb[:],
                                  a2a_out.ap().rearrange("(k p) n -> p k n", p=128))
                if dbg:
                    nc.sync.dma_start(dbg_oall[:, :],
                                      oall_sb[:].rearrange("p k n -> p (k n)"))
                wout_view = wout_d.ap().rearrange("(k p) n -> p k n", p=128)
                wo_tiles = []
                for kt in range(KT):
                    wo_sb = p3.tile([128, DM], F32R, tag=f"wo{kt}")
                    nc.sync.dma_start(wo_sb[:], wout_view[:, kt, :])
                    wo_tiles.append(wo_sb)
                for rw in range(4):
                    y_ps = ps3.tile([128, DM], F32, tag="y")
                    for kt in range(KT):
                        st, sp = kt == 0, kt == KT - 1
                        for nb in range(2):
                            nc.tensor.matmul(
                                y_ps[:, nb * 512:(nb + 1) * 512],
                                oall_sb[:, kt, rw * 128:(rw + 1) * 128],
                                wo_tiles[kt][:, nb * 512:(nb + 1) * 512],
                                start=st, stop=sp)
                    y_sb = p3.tile([128, DM], F32, tag="y_sb")
                    nc.vector.tensor_add(y_sb[:], y_ps[:], boutb_sb[:])
                    nc.sync.dma_start(out_d[rw * 128:(rw + 1) * 128, :], y_sb[:])

    nc.compile()
    return nc


_NC_CACHE = None


def kernel(x, mask, pos_emb, Wq, Wkv, Wout, bout):
    global LAST_EXEC_TIME_NS, LAST_TRACE_DIR, _NC_CACHE

    x = np.asarray(x, dtype=np.float32)
    mask = np.asarray(mask)
    pos_emb = np.asarray(pos_emb, dtype=np.float32)
    Wq = np.asarray(Wq, dtype=np.float32)
    Wkv = np.asarray(Wkv, dtype=np.float32)
    Wout = np.asarray(Wout, dtype=np.float32)
    bout = np.asarray(bout, dtype=np.float32)

    xt = np.ascontiguousarray(x.reshape(R, DM).T)
    wk_full = Wkv[:, :H * DH]
    wv_full = Wkv[:, H * DH:]
    cost = np.ascontiguousarray(np.tile(np.cos(pos_emb).T, (HPC, 1)))
    sint = np.ascontiguousarray(np.tile(np.sin(pos_emb).T, (HPC, 1)))
    maskb = np.ascontiguousarray(
        np.where(mask.reshape(R), 0.0, -1e5).astype(np.float32)
        .reshape(R // 128, 128).T)
    boutb = np.ascontiguousarray(
        np.broadcast_to(bout[None, :], (128, DM)).astype(np.float32))

    in_maps = []
    for c in range(NCORES):
        cols = slice(c * CPC, (c + 1) * CPC)
        in_maps.append({
            "xt": xt,
            "wq": np.ascontiguousarray(Wq[:, cols]),
            "wqr": np.ascontiguousarray(_rot_cols(Wq)[:, cols]),
            "wk": np.ascontiguousarray(wk_full[:, cols]),
            "wkr": np.ascontiguousarray(_rot_cols(wk_full)[:, cols]),
            "wv": np.ascontiguousarray(wv_full[:, cols]),
            "wout": Wout,
            "boutb": boutb,
            "cost": cost,
            "sint": sint,
            "maskb": maskb,
            "vones": np.ones((128, (R // 128) * 2), dtype=np.float32),
        })

    dbg = bool(int(os.environ.get("BASS_KERNEL_DEBUG", "0")))
    if _NC_CACHE is None:
        _NC_CACHE = build(dbg=dbg)
    nc = _NC_CACHE

    trace = bool(int(os.environ.get("BASS_KERNEL_TRACE", "0")))
    kwargs = {}
    if trace:
        _install_trace_shim()
        tdir = os.environ.get("BASS_TRACE_DIR", "/tmp/bass_trace_out")
        os.makedirs(tdir, exist_ok=True)
        kwargs["tmpdir"] = tdir
    res = bass_utils.run_bass_kernel_spmd(
        nc, in_maps, core_ids=list(range(NCORES)), trace=trace, **kwargs)
    LAST_EXEC_TIME_NS = res.exec_time_ns
    if res.instructions_and_trace is not None:
        LAST_TRACE_DIR = res.instructions_and_trace[1]
        globals()["LAST_INSTS"] = res.instructions_and_trace[0]

    globals()["LAST_RESULTS"] = res.results
    y = np.concatenate([res.results[c]["out"] for c in range(NCORES)], axis=0)
    return y.reshape(B, N, DM)


# revision 29
# speedup vs baseline: 1.2926x; 1.2926x over previous
"""Distributed multi-head attention kernel for 8 TRN2 NeuronCores.

Module: B=2, N=2048, D_MODEL=1024, H=16, D_HEAD=64 attention with
arbitrary (non-paired-frequency) rotary embedding, key-side boolean
masking, softmax, and output projection.

Sharding: head-parallel attention (2 heads per core, both batches),
then one AllToAll (~1 MB/core, bf16) to switch to row-parallel for the
output projection. Each core returns a [512, 1024] row block.

Layout / math notes:
 - Matmuls run in bf16 (fp32 PSUM accumulation). bf16 (unlike
   fp32/float32r) lets the PE pipeline LDWEIGHTS behind matmuls.
   Verified end-to-end precision ~5e-3 vs the fp32 reference.
 - qT/kT are produced directly in [chan, row] layout so
   scores = kT_tile.T @ qT come out as scoresT [keys, qrows].
 - Rotary: rotate_every_two(x @ W) == x @ Wr with Wr a column
   permutation/negation of W (host-prepped), so
   q' = (x@Wq)*cos + (x@Wqr)*sin — an extra matmul chain + 3 DVE ops.
 - Key masking is folded into the softmax exp as a per-partition
   (per-key) bias: exp(scale*score + (mask ? 0 : -1e5)).
 - Softmax denominator comes from a ones-column appended to V
   (lhsT = [v_head | 1], M=65); normalization happens after the
   AllToAll (denominators travel in the same A2A buffer), where each
   core normalizes the full channel dim for its row block.
 - PSUM bank discipline: start=True clears the whole bank, so exactly
   one matmul per bank per accumulation chain carries start=True.
"""
import os
import warnings

warnings.filterwarnings("ignore")
import numpy as np
import ml_dtypes

from concourse import bacc, tile, mybir, bass_utils

B, N, DM, H, DH = 2, 2048, 1024, 16, 64
R = B * N            # 4096 flattened rows
NCORES = 8
HPC = 2              # heads per core
CPC = HPC * DH       # chans per core = 128
KT = 8               # contraction tiles over d_model
RB = 8               # row blocks of 512 over R
NKEYT = 16           # key tiles of 128 over N
ROWS_PER_CORE = R // NCORES  # 512
QH = 1024            # qrow half size processed per phase-2 inner pass

F32 = mybir.dt.float32
BF16 = mybir.dt.bfloat16

# A2A row layout: per target shard, 128 chans + 2 denominator rows
SHARD_ROWS = CPC + HPC  # 130

LAST_EXEC_TIME_NS = None
LAST_TRACE_DIR = None


def _install_trace_shim():
    """Register an antenv.axon_hooks NTFF-profile shim (missing in this
    image) so run_bass_kernel_spmd(trace=True) can capture HW profiles."""
    import sys
    import types
    import ctypes
    import contextlib

    if "antenv.axon_hooks" in sys.modules:
        return
    so_path = "/opt/axon/libaxon_pjrt.so"
    hook = None
    if os.path.exists(so_path):
        lib = ctypes.CDLL(so_path)
        if hasattr(lib, "axon_start_nrt_profile"):
            lib.axon_start_nrt_profile.argtypes = [
                ctypes.POINTER(ctypes.c_int64), ctypes.c_size_t]
            lib.axon_start_nrt_profile.restype = ctypes.c_int64
            lib.axon_stop_nrt_profile.argtypes = [ctypes.c_char_p]
            lib.axon_stop_nrt_profile.restype = ctypes.c_int64

            @contextlib.contextmanager
            def _hook(output_dir, device_ids):
                import jax
                jax.devices()
                if device_ids:
                    ids = (ctypes.c_int64 * len(device_ids))(*device_ids)
                    rc = lib.axon_start_nrt_profile(ids, len(device_ids))
                else:
                    rc = lib.axon_start_nrt_profile(None, 0)
                if rc != 0:
                    raise RuntimeError(f"axon_start_nrt_profile rc={rc}")
                try:
                    yield
                finally:
                    n = lib.axon_stop_nrt_profile(str(output_dir).encode())
                    print(f"[trace] {n} profile file(s) -> {output_dir}")

            hook = _hook

    mod = types.ModuleType("antenv.axon_hooks")
    mod.get_axon_ntff_profile_hook = lambda: hook
    mod.set_axon_ntff_profile_hook = lambda h: None
    sys.modules["antenv.axon_hooks"] = mod
    bass_utils.upload_artifacts = lambda tmpdir: tmpdir


def _rot_cols(w):
    """rotate_every_two on output cols: wr[:,2i] = -w[:,2i+1], wr[:,2i+1] = w[:,2i]."""
    wr = np.empty_like(w)
    wr[:, 0::2] = -w[:, 1::2]
    wr[:, 1::2] = w[:, 0::2]
    return wr


def build(dbg=False):
    nc = bacc.Bacc("TRN2", target_bir_lowering=False, debug=False,
                   num_devices=NCORES)

    xt_d = nc.dram_tensor("xt", [DM, R], BF16, kind="ExternalInput")
    wq_d = nc.dram_tensor("wq", [DM, CPC], BF16, kind="ExternalInput")
    wqr_d = nc.dram_tensor("wqr", [DM, CPC], BF16, kind="ExternalInput")
    wk_d = nc.dram_tensor("wk", [DM, CPC], BF16, kind="ExternalInput")
    wkr_d = nc.dram_tensor("wkr", [DM, CPC], BF16, kind="ExternalInput")
    wv_d = nc.dram_tensor("wv", [DM, CPC], BF16, kind="ExternalInput")
    wout_d = nc.dram_tensor("wout", [DM, DM], BF16, kind="ExternalInput")
    boutb_d = nc.dram_tensor("boutb", [128, DM], F32, kind="ExternalInput")
    cost_d = nc.dram_tensor("cost", [CPC, N], F32, kind="ExternalInput")
    sint_d = nc.dram_tensor("sint", [CPC, N], F32, kind="ExternalInput")
    maskb_d = nc.dram_tensor("maskb", [128, R // 128], F32, kind="ExternalInput")
    vones_d = nc.dram_tensor("vones", [128, (R // 128) * 2], BF16,
                             kind="ExternalInput")

    out_d = nc.dram_tensor("out", [ROWS_PER_CORE, DM], F32, kind="ExternalOutput")

    a2a_in = nc.dram_tensor("a2a_in", [NCORES * SHARD_ROWS, ROWS_PER_CORE], BF16)
    a2a_out = nc.dram_tensor("a2a_out", [NCORES * SHARD_ROWS, ROWS_PER_CORE], BF16)

    VAUGW = 2 * (DH + 1)  # 130 cols per key tile: [vA | 1 | vB | 1]

    with tile.TileContext(nc) as tc:
        with tc.tile_pool(name="persist", bufs=1) as pp:
            wq_sb = pp.tile([128, KT, CPC], BF16, tag="wq")
            wqr_sb = pp.tile([128, KT, CPC], BF16, tag="wqr")
            wk_sb = pp.tile([128, KT, CPC], BF16, tag="wk")
            wkr_sb = pp.tile([128, KT, CPC], BF16, tag="wkr")
            wv_sb = pp.tile([128, KT, CPC], BF16, tag="wv")
            cost_sb = pp.tile([CPC, N], F32, tag="cost")
            sint_sb = pp.tile([CPC, N], F32, tag="sint")
            maskb_sb = pp.tile([128, R // 128], F32, tag="maskb")
            boutb_sb = pp.tile([128, DM], F32, tag="boutb")
            qt_sb = pp.tile([CPC, R], BF16, tag="qt")
            kt_sb = pp.tile([CPC, R], BF16, tag="kt")
            vaug_sb = pp.tile([128, (R // 128) * VAUGW], BF16, tag="vaug")
            vones_sb = pp.tile([128, (R // 128) * 2], BF16, tag="vones")
            wo_sb = pp.tile([128, KT, DM], BF16, tag="wo")

            def ktview(d):
                return d.ap().rearrange("(k p) n -> p k n", p=128)

            # per-kt weight DMAs so the first matmuls start early
            for kt in range(KT):
                for sb_t, d_t in [(wq_sb, wq_d), (wqr_sb, wqr_d), (wk_sb, wk_d),
                                  (wkr_sb, wkr_d), (wv_sb, wv_d)]:
                    nc.sync.dma_start(sb_t[:, kt, :], ktview(d_t)[:, kt, :])
            nc.sync.dma_start(cost_sb[:], cost_d[:, :])
            nc.sync.dma_start(sint_sb[:], sint_d[:, :])
            nc.sync.dma_start(maskb_sb[:], maskb_d[:, :])
            nc.sync.dma_start(boutb_sb[:], boutb_d[:, :])
            nc.sync.dma_start(vones_sb[:], vones_d[:, :])
            # ones columns of vaug (cols t*130+64, t*130+129)
            ones_view = vaug_sb[:].rearrange("p (t u w) -> p (t u) w",
                                             u=2, w=DH + 1)[:, :, DH]
            nc.sync.dma_start(ones_view, vones_d[:, :])

            xt_view = xt_d.ap().rearrange("(k p) n -> p k n", p=128)
            nc.sync.dma_start(wo_sb[:], wout_d.ap().rearrange(
                "(k p) n -> p k n", p=128))

            if dbg:
                dbg_qt = nc.dram_tensor("dbg_qt", [CPC, R], BF16,
                                        kind="ExternalOutput")
                dbg_kt = nc.dram_tensor("dbg_kt", [CPC, R], BF16,
                                        kind="ExternalOutput")
                dbg_vaug = nc.dram_tensor(
                    "dbg_vaug", [128, (R // 128) * VAUGW], BF16,
                    kind="ExternalOutput")
                dbg_a2a = nc.dram_tensor(
                    "dbg_a2a", [NCORES * SHARD_ROWS, ROWS_PER_CORE], BF16,
                    kind="ExternalOutput")

            # ---- Phase 1: projections + rotary + v_aug ----
            with tc.tile_pool(name="p1", bufs=2) as p1, \
                 tc.tile_pool(name="ps1", bufs=1, space="PSUM") as ps1:
                for rb in range(RB):
                    c0 = rb * 512
                    xt_sb = p1.tile([128, KT, 512], BF16, tag="xt")
                    for kt in range(KT):
                        nc.sync.dma_start(xt_sb[:, kt, :],
                                          xt_view[:, kt, c0:c0 + 512])

                    q_ps = ps1.tile([128, 512], F32, tag="q")
                    qr_ps = ps1.tile([128, 512], F32, tag="qr")
                    k_ps = ps1.tile([128, 512], F32, tag="k")
                    kr_ps = ps1.tile([128, 512], F32, tag="kr")
                    v_ps = ps1.tile([128, 512], F32, tag="v")
                    for kt in range(KT):
                        st, sp = kt == 0, kt == KT - 1
                        nc.tensor.matmul(q_ps[:], wq_sb[:, kt, :], xt_sb[:, kt, :],
                                         start=st, stop=sp)
                        nc.tensor.matmul(qr_ps[:], wqr_sb[:, kt, :], xt_sb[:, kt, :],
                                         start=st, stop=sp)
                        nc.tensor.matmul(k_ps[:], wk_sb[:, kt, :], xt_sb[:, kt, :],
                                         start=st, stop=sp)
                        nc.tensor.matmul(kr_ps[:], wkr_sb[:, kt, :], xt_sb[:, kt, :],
                                         start=st, stop=sp)
                        # v row-major; one start=True per PSUM bank only
                        for vt in range(4):
                            nc.tensor.matmul(
                                v_ps[:, vt * 128:(vt + 1) * 128],
                                xt_sb[:, kt, vt * 128:(vt + 1) * 128],
                                wv_sb[:, kt, :], start=(st and vt == 0), stop=sp)

                    # rotary: q' = q*cos + qrot*sin (cos/sin indexed mod N)
                    cc = c0 % N
                    tmp = p1.tile([128, 512], BF16, tag="rottmp")
                    for dst, a_ps, b_ps in [(qt_sb, q_ps, qr_ps),
                                            (kt_sb, k_ps, kr_ps)]:
                        dv = dst[:, c0:c0 + 512]
                        nc.vector.tensor_mul(dv, a_ps[:], cost_sb[:, cc:cc + 512])
                        nc.vector.tensor_mul(tmp[:], b_ps[:], sint_sb[:, cc:cc + 512])
                        nc.vector.tensor_add(dv, dv, tmp[:])

                    # v_aug: copy v rows into [vA | 1 | vB | 1] layout
                    kt0 = rb * 4
                    va = vaug_sb[:].rearrange("p (t w) -> p t w", w=VAUGW)
                    vp = v_ps[:].rearrange("p (t c) -> p t c", c=128)
                    nc.vector.tensor_copy(va[:, kt0:kt0 + 4, 0:DH],
                                          vp[:, :, 0:DH])
                    nc.vector.tensor_copy(va[:, kt0:kt0 + 4, DH + 1:DH + 1 + DH],
                                          vp[:, :, DH:2 * DH])

            if dbg:
                nc.sync.dma_start(dbg_qt[:, :], qt_sb[:])
                nc.sync.dma_start(dbg_kt[:, :], kt_sb[:])
                nc.sync.dma_start(dbg_vaug[:, :], vaug_sb[:])

            # ---- Phase 2: attention per (batch, head, qrow-half) ----
            with tc.tile_pool(name="p2", bufs=2) as p2, \
                 tc.tile_pool(name="ps_sc", bufs=2, space="PSUM") as ps_sc, \
                 tc.tile_pool(name="ps_o", bufs=2, space="PSUM") as ps_o:
                for b in range(B):
                    for h in range(HPC):
                        ho = h * DH
                        for qh in range(N // QH):
                            qbase = b * N + qh * QH
                            o_ps = ps_o.tile([DH + 1, QH], F32, tag="outp")
                            for kt in range(NKEYT):
                                g = b * NKEYT + kt
                                krow = b * N + kt * 128
                                sc_ps = ps_sc.tile([128, QH], F32, tag="sc")
                                for qq in range(QH // 512):
                                    nc.tensor.matmul(
                                        sc_ps[:, qq * 512:(qq + 1) * 512],
                                        kt_sb[ho:ho + DH, krow:krow + 128],
                                        qt_sb[ho:ho + DH,
                                              qbase + qq * 512:qbase + (qq + 1) * 512],
                                        start=True, stop=True)
                                p_sb = p2.tile([128, QH], BF16, tag="p")
                                nc.scalar.activation(
                                    p_sb[:], sc_ps[:],
                                    mybir.ActivationFunctionType.Exp,
                                    bias=maskb_sb[:, g:g + 1],
                                    scale=float(DH ** -0.5))
                                va_l = vaug_sb[:, g * VAUGW + h * (DH + 1):
                                               g * VAUGW + h * (DH + 1) + DH + 1]
                                for qq in range(QH // 512):
                                    nc.tensor.matmul(
                                        o_ps[:, qq * 512:(qq + 1) * 512],
                                        va_l,
                                        p_sb[:, qq * 512:(qq + 1) * 512],
                                        start=(kt == 0), stop=(kt == NKEYT - 1))

                            # ship unnormalized out + denominator to A2A buf
                            onb = p2.tile([DH, QH], BF16, tag="onb")
                            denb = p2.tile([1, QH], BF16, tag="denb")
                            nc.vector.tensor_copy(onb[:], o_ps[0:DH, :])
                            nc.vector.tensor_copy(denb[:], o_ps[DH:DH + 1, :])
                            for u in range(QH // 512):
                                jj = qh * (QH // 512) + u
                                j = b * 4 + jj
                                r0 = j * SHARD_ROWS
                                nc.sync.dma_start(
                                    a2a_in[r0 + ho: r0 + ho + DH, :],
                                    onb[:, u * 512:(u + 1) * 512])
                                nc.sync.dma_start(
                                    a2a_in[r0 + CPC + h: r0 + CPC + h + 1, :],
                                    denb[:, u * 512:(u + 1) * 512])

            # ---- Phase 3: AllToAll + normalize + output projection ----
            nc.gpsimd.collective_compute(
                "AllToAll", mybir.AluOpType.bypass,
                replica_groups=[list(range(NCORES))],
                ins=[a2a_in.ap().opt()],
                outs=[a2a_out.ap().opt()])
            if dbg:
                nc.sync.dma_start(dbg_a2a[:, :], a2a_out[:, :])

            with tc.tile_pool(name="p3", bufs=1) as p3, \
                 tc.tile_pool(name="p3b", bufs=2) as p3b, \
                 tc.tile_pool(name="ps3", bufs=2, space="PSUM") as ps3:
                oall_sb = p3.tile([128, KT, 512], BF16, tag="oall")
                a2a_view = a2a_out.ap().rearrange("(j q) n -> q j n", q=SHARD_ROWS)
                nc.sync.dma_start(oall_sb[:], a2a_view[0:CPC, :, :])
                # den_sb partition p = h*8 + j (u-major so the SBUF AP is a
                # single contiguous partition run; a partition-split SBUF AP
                # corrupts the DMA)
                den_sb = p3.tile([2 * NCORES, 512], BF16, tag="den")
                nc.sync.dma_start(den_sb[:], a2a_view[CPC:CPC + 2, :, :])
                recip_sb = p3.tile([2 * NCORES, 512], F32, tag="recip")
                nc.vector.reciprocal(recip_sb[:], den_sb[:])
                # partition_broadcast needs its source at partition 0:
                # gather the 16 recip rows into one partition via DMA
                rrow = p3.tile([1, 2 * NCORES * 512], F32, tag="rrow")
                nc.sync.dma_start(
                    rrow[:].rearrange("p (u n) -> p u n", n=512), recip_sb[:])

                if dbg:
                    dbg_recip = nc.dram_tensor("dbg_recip", [2 * NCORES, 512],
                                               F32, kind="ExternalOutput")
                    dbg_rrow = nc.dram_tensor("dbg_rrow", [1, 2 * NCORES * 512],
                                              F32, kind="ExternalOutput")
                    nc.sync.dma_start(dbg_recip[:, :], recip_sb[:])
                    nc.sync.dma_start(dbg_rrow[:, :], rrow[:])
                    dbg_div = nc.dram_tensor("dbg_div", [128, KT * 512], F32,
                                             kind="ExternalOutput")
                    dbg_onorm = nc.dram_tensor("dbg_onorm", [128, KT * 512],
                                               BF16, kind="ExternalOutput")

                # broadcast ALL 16 recip rows to every partition in one go
                # (partition_broadcast mishandles nonzero free offsets on its
                # input, so a single offset-0 broadcast + per-head column
                # slices in the multiply)
                divall = p3.tile([128, 2 * NCORES * 512], F32, tag="divall")
                nc.gpsimd.partition_broadcast(divall[:], rrow[:])
                onorm_sb = p3.tile([128, KT, 512], BF16, tag="onorm")
                for kt in range(KT):
                    # recip for (peer kt, head h) sits at rrow block h*8+kt
                    nc.vector.tensor_mul(
                        onorm_sb[0:DH, kt, :], oall_sb[0:DH, kt, :],
                        divall[0:DH, kt * 512:(kt + 1) * 512])
                    nc.vector.tensor_mul(
                        onorm_sb[DH:CPC, kt, :], oall_sb[DH:CPC, kt, :],
                        divall[DH:CPC, (NCORES + kt) * 512:(NCORES + kt + 1) * 512])
                    if dbg:
                        nc.sync.dma_start(dbg_div[:, kt * 512:(kt + 1) * 512],
                                          divall[:, kt * 1024:kt * 1024 + 512])
                        nc.sync.dma_start(dbg_onorm[:, kt * 512:(kt + 1) * 512],
                                          onorm_sb[:, kt, :])

                for rw in range(4):
                    y_ps = ps3.tile([128, DM], F32, tag="y")
                    for kt in range(KT):
                        st, sp = kt == 0, kt == KT - 1
                        for nb in range(2):
                            nc.tensor.matmul(
                                y_ps[:, nb * 512:(nb + 1) * 512],
                                onorm_sb[:, kt, rw * 128:(rw + 1) * 128],
                                wo_sb[:, kt, nb * 512:(nb + 1) * 512],
                                start=st, stop=sp)
                    y_sb = p3b.tile([128, DM], F32, tag="y_sb")
                    nc.vector.tensor_add(y_sb[:], y_ps[:], boutb_sb[:])
                    nc.sync.dma_start(out_d[rw * 128:(rw + 1) * 128, :], y_sb[:])

    nc.compile()
    return nc


_NC_CACHE = None


def kernel(x, mask, pos_emb, Wq, Wkv, Wout, bout):
    global LAST_EXEC_TIME_NS, LAST_TRACE_DIR, _NC_CACHE

    x = np.asarray(x, dtype=np.float32)
    mask = np.asarray(mask)
    pos_emb = np.asarray(pos_emb, dtype=np.float32)
    Wq = np.asarray(Wq, dtype=np.float32)
    Wkv = np.asarray(Wkv, dtype=np.float32)
    Wout = np.asarray(Wout, dtype=np.float32)
    bout = np.asarray(bout, dtype=np.float32)

    bf = ml_dtypes.bfloat16
    xt = np.ascontiguousarray(x.reshape(R, DM).T).astype(bf)
    wk_full = Wkv[:, :H * DH]
    wv_full = Wkv[:, H * DH:]
    cost = np.ascontiguousarray(np.tile(np.cos(pos_emb).T, (HPC, 1)))
    sint = np.ascontiguousarray(np.tile(np.sin(pos_emb).T, (HPC, 1)))
    maskb = np.ascontiguousarray(
        np.where(mask.reshape(R), 0.0, -1e5).astype(np.float32)
        .reshape(R // 128, 128).T)
    boutb = np.ascontiguousarray(
        np.broadcast_to(bout[None, :], (128, DM)).astype(np.float32))
    wqr = _rot_cols(Wq)
    wkr = _rot_cols(wk_full)

    in_maps = []
    for c in range(NCORES):
        cols = slice(c * CPC, (c + 1) * CPC)
        in_maps.append({
            "xt": xt,
            "wq": np.ascontiguousarray(Wq[:, cols]).astype(bf),
            "wqr": np.ascontiguousarray(wqr[:, cols]).astype(bf),
            "wk": np.ascontiguousarray(wk_full[:, cols]).astype(bf),
            "wkr": np.ascontiguousarray(wkr[:, cols]).astype(bf),
            "wv": np.ascontiguousarray(wv_full[:, cols]).astype(bf),
            "wout": Wout.astype(bf),
            "boutb": boutb,
            "cost": cost,
            "sint": sint,
            "maskb": maskb,
            "vones": np.ones((128, (R // 128) * 2), dtype=bf),
        })

    dbg = bool(int(os.environ.get("BASS_KERNEL_DEBUG", "0")))
    if _NC_CACHE is None:
        _NC_CACHE = build(dbg=dbg)
    nc = _NC_CACHE

    trace = bool(int(os.environ.get("BASS_KERNEL_TRACE", "0")))
    kwargs = {}
    if trace:
        _install_trace_shim()
        tdir = os.environ.get("BASS_TRACE_DIR", "/tmp/bass_trace_out")
        os.makedirs(tdir, exist_ok=True)
        kwargs["tmpdir"] = tdir
    res = bass_utils.run_bass_kernel_spmd(
        nc, in_maps, core_ids=list(range(NCORES)), trace=trace, **kwargs)
    LAST_EXEC_TIME_NS = res.exec_time_ns
    if res.instructions_and_trace is not None:
        LAST_TRACE_DIR = res.instructions_and_trace[1]
        globals()["LAST_INSTS"] = res.instructions_and_trace[0]

    globals()["LAST_RESULTS"] = res.results
    y = np.concatenate([res.results[c]["out"] for c in range(NCORES)], axis=0)
    return y.reshape(B, N, DM)


# revision 31
# speedup vs baseline: 1.3559x; 1.0490x over previous
"""Distributed multi-head attention kernel for 8 TRN2 NeuronCores.

Module: B=2, N=2048, D_MODEL=1024, H=16, D_HEAD=64 attention with
arbitrary rotary embedding, key-side boolean masking, softmax, and
output projection.

Sharding: head-parallel attention (2 heads per core, both batches),
then one AllToAll (~1 MB/core, bf16) to switch to row-parallel for the
output projection. Each core returns a [512, 1024] row block.

Key design points:
 - All matmuls bf16 with fp32 PSUM accumulation (bf16 lets the PE
   pipeline LDWEIGHTS; fp32/f32r serialize it). End-to-end ~5e-3 rel.
 - qT/kT produced in [chan, row] layout so scores come out transposed
   [keys, qrows] with keys on partitions.
 - Rotary via host-rotated weight copies: rot2(x@W) == x@Wr.
 - Key mask folded into the softmax exp as a per-partition bias.
 - Softmax denominator from a ones-column in V (lhsT = [v | 1], M=65);
   normalization happens after the AllToAll (denominators travel in
   the same buffer: shard layout [hA(64) | denA | hB(64) | denB]).
 - Scores for the two heads issue to PE row groups (0,0)/(64,0) so
   they execute concurrently.
 - One start=True per PSUM bank per accumulation chain (start clears
   the whole bank).
 - DMAs are spread across both HWDGE queues (SP + ACT) and ordered so
   the first projection matmuls start ~10us in.
"""
import os
import warnings

warnings.filterwarnings("ignore")
import numpy as np
import ml_dtypes

from concourse import bacc, tile, mybir, bass_utils

B, N, DM, H, DH = 2, 2048, 1024, 16, 64
R = B * N
NCORES = 8
HPC = 2
CPC = HPC * DH       # 128 chans per core
KT = 8               # contraction tiles over d_model
RB = 8               # row blocks of 512 over R
NKEYT = 16           # key tiles of 128 over N
ROWS_PER_CORE = R // NCORES  # 512
QHS = 1024           # qrows per phase-2 inner pass

F32 = mybir.dt.float32
BF16 = mybir.dt.bfloat16

SHARD_ROWS = CPC + HPC  # 130: [hA 64 | denA 1 | hB 64 | denB 1]

LAST_EXEC_TIME_NS = None
LAST_TRACE_DIR = None


def _install_trace_shim():
    import sys
    import types
    import ctypes
    import contextlib

    if "antenv.axon_hooks" in sys.modules:
        return
    so_path = "/opt/axon/libaxon_pjrt.so"
    hook = None
    if os.path.exists(so_path):
        lib = ctypes.CDLL(so_path)
        if hasattr(lib, "axon_start_nrt_profile"):
            lib.axon_start_nrt_profile.argtypes = [
                ctypes.POINTER(ctypes.c_int64), ctypes.c_size_t]
            lib.axon_start_nrt_profile.restype = ctypes.c_int64
            lib.axon_stop_nrt_profile.argtypes = [ctypes.c_char_p]
            lib.axon_stop_nrt_profile.restype = ctypes.c_int64

            @contextlib.contextmanager
            def _hook(output_dir, device_ids):
                import jax
                jax.devices()
                if device_ids:
                    ids = (ctypes.c_int64 * len(device_ids))(*device_ids)
                    rc = lib.axon_start_nrt_profile(ids, len(device_ids))
                else:
                    rc = lib.axon_start_nrt_profile(None, 0)
                if rc != 0:
                    raise RuntimeError(f"axon_start_nrt_profile rc={rc}")
                try:
                    yield
                finally:
                    n = lib.axon_stop_nrt_profile(str(output_dir).encode())
                    print(f"[trace] {n} profile file(s) -> {output_dir}")

            hook = _hook

    mod = types.ModuleType("antenv.axon_hooks")
    mod.get_axon_ntff_profile_hook = lambda: hook
    mod.set_axon_ntff_profile_hook = lambda h: None
    sys.modules["antenv.axon_hooks"] = mod
    bass_utils.upload_artifacts = lambda tmpdir: tmpdir


def _rot_cols(w):
    wr = np.empty_like(w)
    wr[:, 0::2] = -w[:, 1::2]
    wr[:, 1::2] = w[:, 0::2]
    return wr


def build(dbg=False):
    nc = bacc.Bacc("TRN2", target_bir_lowering=False, debug=False,
                   num_devices=NCORES)

    xt_d = nc.dram_tensor("xt", [DM, R], BF16, kind="ExternalInput")
    wq_d = nc.dram_tensor("wq", [DM, CPC], BF16, kind="ExternalInput")
    wqr_d = nc.dram_tensor("wqr", [DM, CPC], BF16, kind="ExternalInput")
    wk_d = nc.dram_tensor("wk", [DM, CPC], BF16, kind="ExternalInput")
    wkr_d = nc.dram_tensor("wkr", [DM, CPC], BF16, kind="ExternalInput")
    wv_d = nc.dram_tensor("wv", [DM, CPC], BF16, kind="ExternalInput")
    wout_d = nc.dram_tensor("wout", [DM, DM], BF16, kind="ExternalInput")
    boutb_d = nc.dram_tensor("boutb", [128, DM], F32, kind="ExternalInput")
    cost_d = nc.dram_tensor("cost", [CPC, N], F32, kind="ExternalInput")
    sint_d = nc.dram_tensor("sint", [CPC, N], F32, kind="ExternalInput")
    maskb_d = nc.dram_tensor("maskb", [128, R // 128], F32, kind="ExternalInput")
    vones_d = nc.dram_tensor("vones", [128, (R // 128) * 2], BF16,
                             kind="ExternalInput")

    out_d = nc.dram_tensor("out", [ROWS_PER_CORE, DM], F32, kind="ExternalOutput")

    a2a_in = nc.dram_tensor("a2a_in", [NCORES * SHARD_ROWS, ROWS_PER_CORE], BF16)
    a2a_out = nc.dram_tensor("a2a_out", [NCORES * SHARD_ROWS, ROWS_PER_CORE], BF16)

    VAUGW = 2 * (DH + 1)  # 130 cols per key tile: [vA | 1 | vB | 1]

    with tile.TileContext(nc) as tc:
        with tc.tile_pool(name="persist", bufs=1) as pp:
            wq_sb = pp.tile([128, KT, CPC], BF16, tag="wq")
            wqr_sb = pp.tile([128, KT, CPC], BF16, tag="wqr")
            wk_sb = pp.tile([128, KT, CPC], BF16, tag="wk")
            wkr_sb = pp.tile([128, KT, CPC], BF16, tag="wkr")
            wv_sb = pp.tile([128, KT, CPC], BF16, tag="wv")
            cost_sb = pp.tile([CPC, N], F32, tag="cost")
            sint_sb = pp.tile([CPC, N], F32, tag="sint")
            maskb_sb = pp.tile([128, R // 128], F32, tag="maskb")
            boutb_sb = pp.tile([128, DM], F32, tag="boutb")
            qt_sb = pp.tile([CPC, R], BF16, tag="qt")
            kt_sb = pp.tile([CPC, R], BF16, tag="kt")
            vaug_sb = pp.tile([128, (R // 128) * VAUGW], BF16, tag="vaug")
            wo_sb = pp.tile([128, KT, DM], BF16, tag="wo")

            def ktview(d):
                return d.ap().rearrange("(k p) n -> p k n", p=128)

            xt_view = xt_d.ap().rearrange("(k p) n -> p k n", p=128)

            # weights first (alternate queues), then everything else
            for kt in range(KT):
                eng = nc.sync if kt % 2 == 0 else nc.scalar
                for sb_t, d_t in [(wq_sb, wq_d), (wqr_sb, wqr_d), (wk_sb, wk_d),
                                  (wkr_sb, wkr_d), (wv_sb, wv_d)]:
                    eng.dma_start(sb_t[:, kt, :], ktview(d_t)[:, kt, :])
            nc.scalar.dma_start(maskb_sb[:], maskb_d[:, :])
            nc.scalar.dma_start(cost_sb[:], cost_d[:, :])
            nc.scalar.dma_start(sint_sb[:], sint_d[:, :])
            ones_view = vaug_sb[:].rearrange("p (t u w) -> p (t u) w",
                                             u=2, w=DH + 1)[:, :, DH]
            nc.scalar.dma_start(ones_view, vones_d[:, :])

            if dbg:
                dbg_a2a = nc.dram_tensor(
                    "dbg_a2a", [NCORES * SHARD_ROWS, ROWS_PER_CORE], BF16,
                    kind="ExternalOutput")

            # ---- Phase 1: projections + rotary + v_aug ----
            with tc.tile_pool(name="p1", bufs=2) as p1, \
                 tc.tile_pool(name="ps1", bufs=1, space="PSUM") as ps1:
                for rb in range(RB):
                    c0 = rb * 512
                    xt_sb = p1.tile([128, KT, 512], BF16, tag="xt")
                    eng = nc.sync if rb % 2 == 0 else nc.scalar
                    eng.dma_start(xt_sb[:], xt_view[:, :, c0:c0 + 512])

                    q_ps = ps1.tile([128, 512], F32, tag="q")
                    qr_ps = ps1.tile([128, 512], F32, tag="qr")
                    k_ps = ps1.tile([128, 512], F32, tag="k")
                    kr_ps = ps1.tile([128, 512], F32, tag="kr")
                    v_ps = ps1.tile([128, 512], F32, tag="v")
                    for kt in range(KT):
                        st, sp = kt == 0, kt == KT - 1
                        nc.tensor.matmul(q_ps[:], wq_sb[:, kt, :], xt_sb[:, kt, :],
                                         start=st, stop=sp)
                        nc.tensor.matmul(qr_ps[:], wqr_sb[:, kt, :], xt_sb[:, kt, :],
                                         start=st, stop=sp)
                        nc.tensor.matmul(k_ps[:], wk_sb[:, kt, :], xt_sb[:, kt, :],
                                         start=st, stop=sp)
                        nc.tensor.matmul(kr_ps[:], wkr_sb[:, kt, :], xt_sb[:, kt, :],
                                         start=st, stop=sp)
                        for vt in range(4):
                            nc.tensor.matmul(
                                v_ps[:, vt * 128:(vt + 1) * 128],
                                xt_sb[:, kt, vt * 128:(vt + 1) * 128],
                                wv_sb[:, kt, :], start=(st and vt == 0), stop=sp)

                    cc = c0 % N
                    tmp = p1.tile([128, 512], BF16, tag="rottmp")
                    for dst, a_ps, b_ps in [(qt_sb, q_ps, qr_ps),
                                            (kt_sb, k_ps, kr_ps)]:
                        dv = dst[:, c0:c0 + 512]
                        nc.vector.tensor_mul(dv, a_ps[:], cost_sb[:, cc:cc + 512])
                        nc.vector.tensor_mul(tmp[:], b_ps[:], sint_sb[:, cc:cc + 512])
                        nc.vector.tensor_add(dv, dv, tmp[:])

                    kt0 = rb * 4
                    va = vaug_sb[:].rearrange("p (t w) -> p t w", w=VAUGW)
                    vp = v_ps[:].rearrange("p (t c) -> p t c", c=128)
                    nc.vector.tensor_copy(va[:, kt0:kt0 + 4, 0:DH],
                                          vp[:, :, 0:DH])
                    nc.vector.tensor_copy(va[:, kt0:kt0 + 4, DH + 1:DH + 1 + DH],
                                          vp[:, :, DH:2 * DH])

            # wout needed only in phase 3 — load it behind phase-1 traffic
            nc.scalar.dma_start(wo_sb[:], wout_d.ap().rearrange(
                "(k p) n -> p k n", p=128))
            nc.sync.dma_start(boutb_sb[:], boutb_d[:, :])

            # ---- Phase 2: attention, two heads packed, per (b, q-half) ----
            with tc.tile_pool(name="p2", bufs=2) as p2, \
                 tc.tile_pool(name="ps_sc", bufs=1, space="PSUM") as ps_sc, \
                 tc.tile_pool(name="ps_o", bufs=1, space="PSUM") as ps_o:
                for b in range(B):
                    for qh in range(N // QHS):
                        qbase = b * N + qh * QHS
                        o_ps = [ps_o.tile([DH + 1, QHS], F32, tag=f"outp{h}",
                                          name=f"ops{h}") for h in range(HPC)]
                        for kt in range(NKEYT):
                            g = b * NKEYT + kt
                            krow = b * N + kt * 128
                            sc = [ps_sc.tile([128, QHS], F32, tag=f"sc{h}",
                                        name=f"sc{h}") for h in range(HPC)]
                            # interleave heads so the PE runs them in
                            # different row groups concurrently
                            for qq in range(QHS // 512):
                                for h in range(HPC):
                                    ho = h * DH
                                    nc.tensor.matmul(
                                        sc[h][:, qq * 512:(qq + 1) * 512],
                                        kt_sb[ho:ho + DH, krow:krow + 128],
                                        qt_sb[ho:ho + DH,
                                              qbase + qq * 512:qbase + (qq + 1) * 512],
                                        start=True, stop=True)
                            p_sb = []
                            for h in range(HPC):
                                pt = p2.tile([128, QHS], BF16, tag=f"p{h}",
                                             name=f"pt{h}")
                                nc.scalar.activation(
                                    pt[:], sc[h][:],
                                    mybir.ActivationFunctionType.Exp,
                                    bias=maskb_sb[:, g:g + 1],
                                    scale=float(DH ** -0.5))
                                p_sb.append(pt)
                            for h in range(HPC):
                                va_l = vaug_sb[:, g * VAUGW + h * (DH + 1):
                                               g * VAUGW + (h + 1) * (DH + 1)]
                                for qq in range(QHS // 512):
                                    nc.tensor.matmul(
                                        o_ps[h][:, qq * 512:(qq + 1) * 512],
                                        va_l,
                                        p_sb[h][:, qq * 512:(qq + 1) * 512],
                                        start=(kt == 0), stop=(kt == NKEYT - 1))

                        # tail: one bf16 copy + two [65, 512] DMAs per head
                        for h in range(HPC):
                            onb = p2.tile([DH + 1, QHS], BF16, tag=f"onb{h}",
                                          name=f"onb{h}")
                            nc.vector.tensor_copy(onb[:], o_ps[h][:])
                            for u in range(QHS // 512):
                                j = b * 4 + qh * (QHS // 512) + u
                                r0 = j * SHARD_ROWS + h * (DH + 1)
                                nc.sync.dma_start(
                                    a2a_in[r0: r0 + DH + 1, :],
                                    onb[:, u * 512:(u + 1) * 512])

            # ---- Phase 3: AllToAll + normalize + output projection ----
            nc.gpsimd.collective_compute(
                "AllToAll", mybir.AluOpType.bypass,
                replica_groups=[list(range(NCORES))],
                ins=[a2a_in.ap().opt()],
                outs=[a2a_out.ap().opt()])
            if dbg:
                nc.sync.dma_start(dbg_a2a[:, :], a2a_out[:, :])

            with tc.tile_pool(name="p3", bufs=1) as p3, \
                 tc.tile_pool(name="p3b", bufs=2) as p3b, \
                 tc.tile_pool(name="ps3", bufs=2, space="PSUM") as ps3:
                oall_sb = p3.tile([128, KT, 512], BF16, tag="oall")
                a2a_view = a2a_out.ap().rearrange("(j q) n -> q j n", q=SHARD_ROWS)
                nc.sync.dma_start(oall_sb[0:DH, :, :], a2a_view[0:DH, :, :])
                nc.scalar.dma_start(oall_sb[DH:CPC, :, :],
                                    a2a_view[DH + 1:CPC + 1, :, :])
                # den_sb partition p = h*8 + j (single contiguous partition run)
                den_sb = p3.tile([2 * NCORES, 512], BF16, tag="den")
                nc.sync.dma_start(den_sb[0:NCORES, :], a2a_view[DH:DH + 1, :, :])
                nc.sync.dma_start(den_sb[NCORES:2 * NCORES, :],
                                  a2a_view[CPC + 1:CPC + 2, :, :])
                recip_sb = p3.tile([2 * NCORES, 512], F32, tag="recip")
                nc.vector.reciprocal(recip_sb[:], den_sb[:])
                # gather recips into partition 0, then broadcast to all 128
                rrow = p3.tile([1, 2 * NCORES * 512], F32, tag="rrow")
                nc.sync.dma_start(
                    rrow[:].rearrange("p (u n) -> p u n", n=512), recip_sb[:])
                divall = p3.tile([128, 2 * NCORES * 512], F32, tag="divall")
                nc.gpsimd.partition_broadcast(divall[:], rrow[:])

                onorm_sb = p3.tile([128, KT, 512], BF16, tag="onorm")
                for kt in range(KT):
                    nc.vector.tensor_mul(
                        onorm_sb[0:DH, kt, :], oall_sb[0:DH, kt, :],
                        divall[0:DH, kt * 512:(kt + 1) * 512])
                    nc.vector.tensor_mul(
                        onorm_sb[DH:CPC, kt, :], oall_sb[DH:CPC, kt, :],
                        divall[DH:CPC, (NCORES + kt) * 512:(NCORES + kt + 1) * 512])

                for rw in range(4):
                    y_ps = ps3.tile([128, DM], F32, tag="y")
                    for kt in range(KT):
                        st, sp = kt == 0, kt == KT - 1
                        for nb in range(2):
                            nc.tensor.matmul(
                                y_ps[:, nb * 512:(nb + 1) * 512],
                                onorm_sb[:, kt, rw * 128:(rw + 1) * 128],
                                wo_sb[:, kt, nb * 512:(nb + 1) * 512],
                                start=st, stop=sp)
                    y_sb = p3b.tile([128, DM], F32, tag="y_sb")
                    nc.vector.tensor_add(y_sb[:], y_ps[:], boutb_sb[:])
                    eng = nc.sync if rw % 2 == 0 else nc.scalar
                    eng.dma_start(out_d[rw * 128:(rw + 1) * 128, :], y_sb[:])

    nc.compile()
    return nc


_NC_CACHE = None


def kernel(x, mask, pos_emb, Wq, Wkv, Wout, bout):
    global LAST_EXEC_TIME_NS, LAST_TRACE_DIR, _NC_CACHE

    x = np.asarray(x, dtype=np.float32)
    mask = np.asarray(mask)
    pos_emb = np.asarray(pos_emb, dtype=np.float32)
    Wq = np.asarray(Wq, dtype=np.float32)
    Wkv = np.asarray(Wkv, dtype=np.float32)
    Wout = np.asarray(Wout, dtype=np.float32)
    bout = np.asarray(bout, dtype=np.float32)

    bf = ml_dtypes.bfloat16
    xt = np.ascontiguousarray(x.reshape(R, DM).T).astype(bf)
    wk_full = Wkv[:, :H * DH]
    wv_full = Wkv[:, H * DH:]
    cost = np.ascontiguousarray(np.tile(np.cos(pos_emb).T, (HPC, 1)))
    sint = np.ascontiguousarray(np.tile(np.sin(pos_emb).T, (HPC, 1)))
    maskb = np.ascontiguousarray(
        np.where(mask.reshape(R), 0.0, -1e5).astype(np.float32)
        .reshape(R // 128, 128).T)
    boutb = np.ascontiguousarray(
        np.broadcast_to(bout[None, :], (128, DM)).astype(np.float32))
    wqr = _rot_cols(Wq)
    wkr = _rot_cols(wk_full)

    in_maps = []
    for c in range(NCORES):
        cols = slice(c * CPC, (c + 1) * CPC)
        in_maps.append({
            "xt": xt,
            "wq": np.ascontiguousarray(Wq[:, cols]).astype(bf),
            "wqr": np.ascontiguousarray(wqr[:, cols]).astype(bf),
            "wk": np.ascontiguousarray(wk_full[:, cols]).astype(bf),
            "wkr": np.ascontiguousarray(wkr[:, cols]).astype(bf),
            "wv": np.ascontiguousarray(wv_full[:, cols]).astype(bf),
            "wout": Wout.astype(bf),
            "boutb": boutb,
            "cost": cost,
            "sint": sint,
            "maskb": maskb,
            "vones": np.ones((128, (R // 128) * 2), dtype=bf),
        })

    dbg = bool(int(os.environ.get("BASS_KERNEL_DEBUG", "0")))
    if _NC_CACHE is None:
        _NC_CACHE = build(dbg=dbg)
    nc = _NC_CACHE

    trace = bool(int(os.environ.get("BASS_KERNEL_TRACE", "0")))
    kwargs = {}
    if trace:
        _install_trace_shim()
        tdir = os.environ.get("BASS_TRACE_DIR", "/tmp/bass_trace_out")
        os.makedirs(tdir, exist_ok=True)
        kwargs["tmpdir"] = tdir
    res = bass_utils.run_bass_kernel_spmd(
        nc, in_maps, core_ids=list(range(NCORES)), trace=trace, **kwargs)
    LAST_EXEC_TIME_NS = res.exec_time_ns
    if res.instructions_and_trace is not None:
        LAST_TRACE_DIR = res.instructions_and_trace[1]
        globals()["LAST_INSTS"] = res.instructions_and_trace[0]

    globals()["LAST_RESULTS"] = res.results
    y = np.concatenate([res.results[c]["out"] for c in range(NCORES)], axis=0)
    return y.reshape(B, N, DM)
